# revision 55
# baseline (speedup 1.0000x reference)
"""Block-sparse attention Trainium2 kernel (8 NeuronCores, SPMD).

Problem: hidden_states [2, 2048, 2048] fp32; Wq/Wk/Wv [2048, 2048]; Wo
[2048, 2048]. 16 heads x 128 dim, block-banded attention (BLOCK=64,
bandwidth 2 -> each 128-query tile attends a 384-key band with two
64x64 invalid corners).

Sharding: core c = (batch b = c//4) x (head group g = c%4, 4 heads).
Each core computes q/k/v projections for its 4 heads (columns of
Wq/Wk/Wv), banded attention, and a partial output through its rows of
Wo. Host sums the 4 partials per batch. No collectives.

Per-core pipeline (all matmuls bf16, fp32 PSUM accumulate; inputs are
pre-transposed/cast to bf16 host-side during sharding):
  hT [hid, seq] + weight tiles stream in over HWDGE/SWDGE (issue spread
  across sync/scalar/gpsimd sequencers; hT halved for early deps).
  QT_h/KT_h produced directly transposed (lhsT=weight slice, rhs=hT);
  V natural [seq, d] (lhsT=hT slice, rhs=Wv).
  scores = QT^T KT band -> +mask tile (fused PSUM->SBUF move) ->
  exp with fused rowsum (no max-subtract; scores are O(+-8)) ->
  reciprocal -> normalize P -> PE-transpose P chunks -> PV -> AO^T bf16.
  out_partial = AO @ Wo_rows via lhsT=AO^T, fused into the last head's
  loop (2-tile lag); bf16 partials summed in fp32 on host.
Measured: ~312 us HW exec (max over 8 cores), ~77% bf16 MFU,
rel err ~6e-3 vs the fp32 reference.
"""

from contextlib import ExitStack

import numpy as np

import concourse.bass as bass
import concourse.mybir as mybir
import concourse.tile as tile
from concourse import bacc
from concourse.bass_utils import run_bass_kernel_spmd
from concourse.masks import make_identity

S = 2048          # sequence length
HID = 2048        # hidden size
HL = 4            # heads per core
D = 128           # head dim
NKT = HID // 128  # 16 contraction tiles
NQ = S // 128     # 16 query tiles
SCALE = float(D) ** -0.5
NEG = -1e30
BF = mybir.dt.bfloat16
F32 = mybir.dt.float32


def _emit_wo(nc, ps_big, osb_pool, AO_T, wo_s, out, mt):
    mts = slice(128 * mt, 128 * (mt + 1))
    for nc_ in range(4):
        ns = slice(512 * nc_, 512 * (nc_ + 1))
        ops_ = ps_big.tile([128, 512], mybir.dt.float32, tag="big", name="wops")
        for dk in range(HL):
            nc.tensor.matmul(
                ops_, lhsT=AO_T[dk][:, mts], rhs=wo_s[dk][:, ns],
                start=(dk == 0), stop=(dk == HL - 1),
            )
        osb = osb_pool.tile([128, 512], BF, tag="osb", name="osb")
        nc.any.tensor_copy(osb, ops_)
        nc.sync.dma_start(out=out[mts, ns], in_=osb)


def build():
    nc = bacc.Bacc()
    # ht = h^T [hidden, seq]; all inputs pre-transposed/cast to bf16
    # host-side during sharding
    ht = nc.declare_dram_parameter("ht", [HID, S], BF, isOutput=False)
    wq = nc.declare_dram_parameter("wq", [HID, HL * D], BF, isOutput=False)
    wk = nc.declare_dram_parameter("wk", [HID, HL * D], BF, isOutput=False)
    wv = nc.declare_dram_parameter("wv", [HID, HL * D], BF, isOutput=False)
    wo = nc.declare_dram_parameter("wo", [HL * D, HID], BF, isOutput=False)
    out = nc.declare_dram_parameter("out", [S, HID], BF, isOutput=True)

    with ExitStack() as ctx:
        tc = ctx.enter_context(tile.TileContext(nc))
        persist = ctx.enter_context(tc.tile_pool(name="persist", bufs=1))
        qk = ctx.enter_context(tc.tile_pool(name="qk", bufs=2))
        work = ctx.enter_context(tc.tile_pool(name="work", bufs=5))
        stats = ctx.enter_context(tc.tile_pool(name="stats", bufs=8))
        osb_pool = ctx.enter_context(tc.tile_pool(name="osb", bufs=3))
        ps_big = ctx.enter_context(tc.tile_pool(name="ps_big", bufs=4, space="PSUM"))
        ps_sc = ctx.enter_context(tc.tile_pool(name="ps_sc", bufs=1, space="PSUM"))
        ps_pt = ctx.enter_context(tc.tile_pool(name="ps_pt", bufs=2, space="PSUM"))
        ps_ao = ctx.enter_context(tc.tile_pool(name="ps_ao", bufs=1, space="PSUM"))

        ident = persist.tile([128, 128], BF, tag="ident")
        make_identity(nc, ident)

        # additive corner masks for the 384-wide (interior) and 256-wide
        # (edge) score bands; built once
        mask_int = persist.tile([128, 384], F32, tag="mask_int")
        nc.gpsimd.memset(mask_int, 0.0)
        nc.gpsimd.memset(mask_int[0:64, 320:384], NEG)
        nc.gpsimd.memset(mask_int[64:128, 0:64], NEG)
        mask_lo = persist.tile([128, 256], F32, tag="mask_lo")
        nc.gpsimd.memset(mask_lo, 0.0)
        nc.gpsimd.memset(mask_lo[0:64, 192:256], NEG)
        mask_hi = persist.tile([128, 256], F32, tag="mask_hi")
        nc.gpsimd.memset(mask_hi, 0.0)
        nc.gpsimd.memset(mask_hi[64:128, 0:64], NEG)

        # HAM warm-up: ~5us of dependency-free matmuls at t=0 flips the
        # PE clock gate to 2.4GHz before the first real projection MMs
        # (which are DMA-paced and would otherwise run the first ~45us
        # at the cold 1.2GHz K=4/8 state)
        warm_ps = ps_ao.tile([128, 128], F32, tag="ao", name="warm_ps")
        for _ in range(200):
            nc.tensor.matmul(warm_ps, lhsT=ident, rhs=ident, start=True, stop=True)

        # ---- input loads (plain HWDGE DMAs, bf16). Order matters: the V
        # projection consumes hT[k] + wv_s[k] first, so those lead.
        hT = [persist.tile([128, S], BF, tag=f"ht{k}", name=f"ht{k}") for k in range(NKT)]
        wq_s = [persist.tile([128, HL * D], BF, tag=f"wq{k}", name=f"wq{k}") for k in range(NKT)]
        wk_s = [persist.tile([128, HL * D], BF, tag=f"wk{k}", name=f"wk{k}") for k in range(NKT)]
        wv_s = [persist.tile([128, HL * D], BF, tag=f"wv{k}", name=f"wv{k}") for k in range(NKT)]
        # half-split hT loads, issue spread across the two HWDGE
        # sequencers (sync + scalar) plus gpsimd for the later weights —
        # DMA issue is ~0.6us per dma_start and serializes per engine
        for k in range(NKT):
            ks = slice(128 * k, 128 * (k + 1))
            nc.sync.dma_start(out=hT[k][:, 0:1024], in_=ht[ks, 0:1024])
            nc.scalar.dma_start(out=wq_s[k], in_=wq[ks, :])
            nc.scalar.dma_start(out=wk_s[k], in_=wk[ks, :])
            # wv rides the slower SWDGE stream: V is consumed ~25us in,
            # while wq/wk gate the very first projection groups
            nc.gpsimd.dma_start(out=wv_s[k], in_=wv[ks, :])
        for k in range(NKT):
            ks = slice(128 * k, 128 * (k + 1))
            nc.sync.dma_start(out=hT[k][:, 1024:2048], in_=ht[ks, 1024:2048])
        wo_s = [persist.tile([128, HID], BF, tag=f"wo{k}", name=f"wo{k}") for k in range(HL)]
        for k in range(HL):
            nc.gpsimd.dma_start(out=wo_s[k], in_=wo[128 * k : 128 * (k + 1), :])

        V = [persist.tile([128, HL * D], BF, tag=f"v{t}", name=f"v{t}") for t in range(NQ)]

        AO_T = [persist.tile([128, S], BF, tag=f"ao{hh}", name=f"ao{hh}") for hh in range(HL)]

        for hh in range(HL):
            hs_ = slice(128 * hh, 128 * (hh + 1))
            QT = qk.tile([128, S], BF, tag="q")
            KT = qk.tile([128, S], BF, tag="k")
            for mc in range(4):
                ms = slice(512 * mc, 512 * (mc + 1))
                qps = ps_big.tile([128, 512], F32, tag="big")
                for k in range(NKT):
                    nc.tensor.matmul(
                        qps, lhsT=wq_s[k][:, hs_], rhs=hT[k][:, ms],
                        start=(k == 0), stop=(k == NKT - 1),
                    )
                # fold the 1/sqrt(d) scaling into Q
                nc.vector.tensor_scalar_mul(QT[:, ms], qps, SCALE)
                kps = ps_big.tile([128, 512], F32, tag="big")
                for k in range(NKT):
                    nc.tensor.matmul(
                        kps, lhsT=wk_s[k][:, hs_], rhs=hT[k][:, ms],
                        start=(k == 0), stop=(k == NKT - 1),
                    )
                nc.vector.tensor_copy(KT[:, ms], kps)

            if hh == 0:
                # V projection, natural layout [seq, 4*128]; placed after
                # head-0 QK so attention can start as early as possible
                for t in range(NQ):
                    vps = ps_big.tile([128, 512], F32, tag="big")
                    ts_ = slice(128 * t, 128 * (t + 1))
                    for k in range(NKT):
                        nc.tensor.matmul(
                            vps, lhsT=hT[k][:, ts_], rhs=wv_s[k],
                            start=(k == 0), stop=(k == NKT - 1),
                        )
                    nc.vector.tensor_copy(V[t], vps)

            for qt in range(NQ):
                t0 = max(0, 128 * qt - 128)
                t1 = min(S, 128 * qt + 256)
                W = t1 - t0
                scps = ps_sc.tile([128, W], F32, tag="sc")
                nc.tensor.matmul(
                    scps, lhsT=QT[:, 128 * qt : 128 * (qt + 1)], rhs=KT[:, t0:t1],
                    start=True, stop=True,
                )
                sc = work.tile([128, W], F32, tag="scsb")
                mask = mask_lo if qt == 0 else (mask_hi if qt == NQ - 1 else mask_int)
                # copy PSUM->SBUF fused with the corner mask add
                nc.vector.tensor_add(sc, scps, mask)
                # scores are O(+-8) so exp needs no max subtraction
                # (softmax is shift-invariant; fp32 exp is safe here)
                p = work.tile([128, W], BF, tag="p")
                rsum = stats.tile([128, 1], F32, tag="rsum")
                nc.scalar.activation(
                    p, sc, mybir.ActivationFunctionType.Exp,
                    bias=0.0, scale=1.0, accum_out=rsum,
                )
                rcp = stats.tile([128, 1], F32, tag="rcp")
                nc.vector.reciprocal(rcp, rsum)
                nc.vector.tensor_scalar_mul(p, p, rcp)
                aops = ps_ao.tile([128, 128], F32, tag="ao")
                nch = W // 128
                for ci in range(nch):
                    ptps = ps_pt.tile([128, 128], BF, tag="pt")
                    nc.tensor.transpose(
                        ptps, p[:, 128 * ci : 128 * (ci + 1)], ident
                    )
                    pts = work.tile([128, 128], BF, tag="pts")
                    if ci % 2 == 0:
                        nc.vector.tensor_copy(pts, ptps)
                    else:
                        nc.scalar.copy(pts, ptps)
                    tt = t0 // 128 + ci
                    nc.tensor.matmul(
                        aops, lhsT=V[tt][:, hs_], rhs=pts,
                        start=(ci == 0), stop=(ci == nch - 1),
                    )
                nc.scalar.copy(AO_T[hh][:, 128 * qt : 128 * (qt + 1)], aops)

                # fuse the output projection into the last head's loop
                # with a 2-tile lag so Wo matmuls are never gated on the
                # in-flight softmax chain of the same tile
                if hh == HL - 1 and qt >= 2:
                    _emit_wo(nc, ps_big, osb_pool, AO_T, wo_s, out, qt - 2)
        for mt in (NQ - 2, NQ - 1):
            _emit_wo(nc, ps_big, osb_pool, AO_T, wo_s, out, mt)

    if not nc.is_finalized():
        nc.finalize()
    return nc


_NC = None


def _get_nc():
    global _NC
    if _NC is None:
        _NC = build()
    return _NC


def _in_maps(hidden_states, Wq, Wk, Wv, Wo):
    import ml_dtypes

    bf = ml_dtypes.bfloat16
    hs = np.asarray(hidden_states, dtype=np.float32)
    Wq = np.asarray(Wq, dtype=np.float32)
    Wk = np.asarray(Wk, dtype=np.float32)
    Wv = np.asarray(Wv, dtype=np.float32)
    Wo = np.asarray(Wo, dtype=np.float32)
    maps = []
    for c in range(8):
        b, g = divmod(c, 4)
        sl = slice(512 * g, 512 * (g + 1))
        maps.append(
            {
                "ht": np.ascontiguousarray(hs[b].T).astype(bf),
                "wq": np.ascontiguousarray(Wq[:, sl]).astype(bf),
                "wk": np.ascontiguousarray(Wk[:, sl]).astype(bf),
                "wv": np.ascontiguousarray(Wv[:, sl]).astype(bf),
                "wo": np.ascontiguousarray(Wo[sl, :]).astype(bf),
            }
        )
    return maps


def _gather(results):
    outs = [np.asarray(results[c]["out"]).astype(np.float32) for c in range(8)]
    return np.stack(
        [outs[0] + outs[1] + outs[2] + outs[3],
         outs[4] + outs[5] + outs[6] + outs[7]]
    )


def run(in_maps, trace=False, **kw):
    nc = _get_nc()
    return run_bass_kernel_spmd(nc, in_maps, core_ids=list(range(8)), trace=trace, **kw)


def kernel(hidden_states, Wq, Wk, Wv, Wo):
    maps = _in_maps(hidden_states, Wq, Wk, Wv, Wo)
    res = run(maps)
    return _gather(res.results)


# revision 58
# speedup vs baseline: 1.0287x; 1.0287x over previous
"""Block-sparse attention Trainium2 kernel (8 NeuronCores, SPMD).

Problem: hidden_states [2, 2048, 2048] fp32; Wq/Wk/Wv [2048, 2048]; Wo
[2048, 2048]. 16 heads x 128 dim, block-banded attention (BLOCK=64,
bandwidth 2 -> each 128-query tile attends a 384-key band with two
64x64 invalid corners).

Sharding: core c = (batch b = c//4) x (head group g = c%4, 4 heads).
Each core computes q/k/v projections for its 4 heads (columns of
Wq/Wk/Wv), banded attention, and a partial output through its rows of
Wo. Host sums the 4 partials per batch. No collectives.

Per-core pipeline (all matmuls bf16, fp32 PSUM accumulate; inputs are
pre-transposed/cast to bf16 host-side during sharding):
  hT [hid, seq] + weight tiles stream in over HWDGE/SWDGE (issue spread
  across sync/scalar/gpsimd sequencers; hT halved for early deps).
  QT_h/KT_h produced directly transposed (lhsT=weight slice, rhs=hT);
  V natural [seq, d] (lhsT=hT slice, rhs=Wv).
  scores = QT^T KT band -> +mask tile (fused PSUM->SBUF move) ->
  exp with fused rowsum (no max-subtract; scores are O(+-8)) ->
  reciprocal -> normalize P -> PE-transpose P chunks -> PV -> AO^T bf16.
  out_partial = AO @ Wo_rows via lhsT=AO^T, fused into the last head's
  loop (2-tile lag); bf16 partials summed in fp32 on host.
Measured: ~312 us HW exec (max over 8 cores), ~77% bf16 MFU,
rel err ~6e-3 vs the fp32 reference.
"""

from contextlib import ExitStack

import numpy as np

import concourse.bass as bass
import concourse.mybir as mybir
import concourse.tile as tile
from concourse import bacc
from concourse.bass_utils import run_bass_kernel_spmd
from concourse.masks import make_identity

S = 2048          # sequence length
HID = 2048        # hidden size
HL = 4            # heads per core
D = 128           # head dim
NKT = HID // 128  # 16 contraction tiles
NQ = S // 128     # 16 query tiles
SCALE = float(D) ** -0.5
NEG = -1e30
BF = mybir.dt.bfloat16
F32 = mybir.dt.float32


def _emit_wo(nc, ps_big, osb_pool, AO_T, wo_s, out, mt):
    mts = slice(128 * mt, 128 * (mt + 1))
    for nc_ in range(4):
        ns = slice(512 * nc_, 512 * (nc_ + 1))
        ops_ = ps_big.tile([128, 512], mybir.dt.float32, tag="big", name="wops")
        for dk in range(HL):
            nc.tensor.matmul(
                ops_, lhsT=AO_T[dk][:, mts], rhs=wo_s[dk][:, ns],
                start=(dk == 0), stop=(dk == HL - 1),
            )
        osb = osb_pool.tile([128, 512], BF, tag="osb", name="osb")
        nc.any.tensor_copy(osb, ops_)
        nc.sync.dma_start(out=out[mts, ns], in_=osb)


def build():
    nc = bacc.Bacc()
    # ht = h^T [hidden, seq]; all inputs pre-transposed/cast to bf16
    # host-side during sharding
    ht = nc.declare_dram_parameter("ht", [HID, S], BF, isOutput=False)
    wq = nc.declare_dram_parameter("wq", [HID, HL * D], BF, isOutput=False)
    wk = nc.declare_dram_parameter("wk", [HID, HL * D], BF, isOutput=False)
    wv = nc.declare_dram_parameter("wv", [HID, HL * D], BF, isOutput=False)
    wo = nc.declare_dram_parameter("wo", [HL * D, HID], BF, isOutput=False)
    out = nc.declare_dram_parameter("out", [S, HID], BF, isOutput=True)

    with ExitStack() as ctx:
        tc = ctx.enter_context(tile.TileContext(nc))
        persist = ctx.enter_context(tc.tile_pool(name="persist", bufs=1))
        qk = ctx.enter_context(tc.tile_pool(name="qk", bufs=2))
        work = ctx.enter_context(tc.tile_pool(name="work", bufs=5))
        stats = ctx.enter_context(tc.tile_pool(name="stats", bufs=8))
        osb_pool = ctx.enter_context(tc.tile_pool(name="osb", bufs=3))
        ps_big = ctx.enter_context(tc.tile_pool(name="ps_big", bufs=4, space="PSUM"))
        ps_sc = ctx.enter_context(tc.tile_pool(name="ps_sc", bufs=1, space="PSUM"))
        ps_pt = ctx.enter_context(tc.tile_pool(name="ps_pt", bufs=2, space="PSUM"))
        ps_ao = ctx.enter_context(tc.tile_pool(name="ps_ao", bufs=1, space="PSUM"))

        ident = persist.tile([128, 128], BF, tag="ident")
        make_identity(nc, ident)

        # additive corner masks for the 384-wide (interior) and 256-wide
        # (edge) score bands; built once
        mask_int = persist.tile([128, 384], F32, tag="mask_int")
        nc.gpsimd.memset(mask_int, 0.0)
        nc.gpsimd.memset(mask_int[0:64, 320:384], NEG)
        nc.gpsimd.memset(mask_int[64:128, 0:64], NEG)
        mask_lo = persist.tile([128, 256], F32, tag="mask_lo")
        nc.gpsimd.memset(mask_lo, 0.0)
        nc.gpsimd.memset(mask_lo[0:64, 192:256], NEG)
        mask_hi = persist.tile([128, 256], F32, tag="mask_hi")
        nc.gpsimd.memset(mask_hi, 0.0)
        nc.gpsimd.memset(mask_hi[64:128, 0:64], NEG)

        # HAM warm-up: ~5us of dependency-free matmuls at t=0 flips the
        # PE clock gate to 2.4GHz before the first real projection MMs
        # (which are DMA-paced and would otherwise run the first ~45us
        # at the cold 1.2GHz K=4/8 state)
        warm_ps = ps_ao.tile([128, 128], F32, tag="ao", name="warm_ps")
        for _ in range(48):
            nc.tensor.matmul(warm_ps, lhsT=ident, rhs=ident, start=True, stop=True)

        # ---- input loads (plain HWDGE DMAs, bf16). Order matters: the V
        # projection consumes hT[k] + wv_s[k] first, so those lead.
        hT = [persist.tile([128, S], BF, tag=f"ht{k}", name=f"ht{k}") for k in range(NKT)]
        wq_s = [persist.tile([128, HL * D], BF, tag=f"wq{k}", name=f"wq{k}") for k in range(NKT)]
        wk_s = [persist.tile([128, HL * D], BF, tag=f"wk{k}", name=f"wk{k}") for k in range(NKT)]
        wv_s = [persist.tile([128, HL * D], BF, tag=f"wv{k}", name=f"wv{k}") for k in range(NKT)]
        # half-split hT loads, issue spread across the two HWDGE
        # sequencers (sync + scalar) plus gpsimd for the later weights —
        # DMA issue is ~0.6us per dma_start and serializes per engine
        for k in range(NKT):
            ks = slice(128 * k, 128 * (k + 1))
            nc.sync.dma_start(out=hT[k][:, 0:1024], in_=ht[ks, 0:1024])
            nc.scalar.dma_start(out=wq_s[k], in_=wq[ks, :])
            nc.scalar.dma_start(out=wk_s[k], in_=wk[ks, :])
            # wv rides the slower SWDGE stream: V is consumed ~25us in,
            # while wq/wk gate the very first projection groups
            nc.gpsimd.dma_start(out=wv_s[k], in_=wv[ks, :])
        for k in range(NKT):
            ks = slice(128 * k, 128 * (k + 1))
            nc.sync.dma_start(out=hT[k][:, 1024:2048], in_=ht[ks, 1024:2048])
        wo_s = [persist.tile([128, HID], BF, tag=f"wo{k}", name=f"wo{k}") for k in range(HL)]
        for k in range(HL):
            nc.gpsimd.dma_start(out=wo_s[k], in_=wo[128 * k : 128 * (k + 1), :])

        V = [persist.tile([128, HL * D], BF, tag=f"v{t}", name=f"v{t}") for t in range(NQ)]

        AO_T = [persist.tile([128, S], BF, tag=f"ao{hh}", name=f"ao{hh}") for hh in range(HL)]

        for hh in range(HL):
            hs_ = slice(128 * hh, 128 * (hh + 1))
            QT = qk.tile([128, S], BF, tag="q")
            KT = qk.tile([128, S], BF, tag="k")
            for mc in range(4):
                ms = slice(512 * mc, 512 * (mc + 1))
                qps = ps_big.tile([128, 512], F32, tag="big")
                for k in range(NKT):
                    nc.tensor.matmul(
                        qps, lhsT=wq_s[k][:, hs_], rhs=hT[k][:, ms],
                        start=(k == 0), stop=(k == NKT - 1),
                    )
                # fold the 1/sqrt(d) scaling into Q
                nc.vector.tensor_scalar_mul(QT[:, ms], qps, SCALE)
                kps = ps_big.tile([128, 512], F32, tag="big")
                for k in range(NKT):
                    nc.tensor.matmul(
                        kps, lhsT=wk_s[k][:, hs_], rhs=hT[k][:, ms],
                        start=(k == 0), stop=(k == NKT - 1),
                    )
                nc.vector.tensor_copy(KT[:, ms], kps)

            if hh == 0:
                # V projection, natural layout [seq, 4*128]; placed after
                # head-0 QK so attention can start as early as possible
                for t in range(NQ):
                    vps = ps_big.tile([128, 512], F32, tag="big")
                    ts_ = slice(128 * t, 128 * (t + 1))
                    for k in range(NKT):
                        nc.tensor.matmul(
                            vps, lhsT=hT[k][:, ts_], rhs=wv_s[k],
                            start=(k == 0), stop=(k == NKT - 1),
                        )
                    nc.vector.tensor_copy(V[t], vps)

            for qt in range(NQ):
                t0 = max(0, 128 * qt - 128)
                t1 = min(S, 128 * qt + 256)
                W = t1 - t0
                scps = ps_sc.tile([128, W], F32, tag="sc")
                nc.tensor.matmul(
                    scps, lhsT=QT[:, 128 * qt : 128 * (qt + 1)], rhs=KT[:, t0:t1],
                    start=True, stop=True,
                )
                sc = work.tile([128, W], F32, tag="scsb")
                mask = mask_lo if qt == 0 else (mask_hi if qt == NQ - 1 else mask_int)
                # copy PSUM->SBUF fused with the corner mask add
                nc.vector.tensor_add(sc, scps, mask)
                # scores are O(+-8) so exp needs no max subtraction
                # (softmax is shift-invariant; fp32 exp is safe here)
                p = work.tile([128, W], BF, tag="p")
                rsum = stats.tile([128, 1], F32, tag="rsum")
                nc.scalar.activation(
                    p, sc, mybir.ActivationFunctionType.Exp,
                    bias=0.0, scale=1.0, accum_out=rsum,
                )
                rcp = stats.tile([128, 1], F32, tag="rcp")
                nc.vector.reciprocal(rcp, rsum)
                nc.vector.tensor_scalar_mul(p, p, rcp)
                aops = ps_ao.tile([128, 128], F32, tag="ao")
                nch = W // 128
                for ci in range(nch):
                    ptps = ps_pt.tile([128, 128], BF, tag="pt")
                    nc.tensor.transpose(
                        ptps, p[:, 128 * ci : 128 * (ci + 1)], ident
                    )
                    pts = work.tile([128, 128], BF, tag="pts")
                    if ci % 2 == 0:
                        nc.vector.tensor_copy(pts, ptps)
                    else:
                        nc.scalar.copy(pts, ptps)
                    tt = t0 // 128 + ci
                    nc.tensor.matmul(
                        aops, lhsT=V[tt][:, hs_], rhs=pts,
                        start=(ci == 0), stop=(ci == nch - 1),
                    )
                nc.scalar.copy(AO_T[hh][:, 128 * qt : 128 * (qt + 1)], aops)

                # fuse the output projection into the last head's loop
                # with a 2-tile lag so Wo matmuls are never gated on the
                # in-flight softmax chain of the same tile
                if hh == HL - 1 and qt >= 2:
                    _emit_wo(nc, ps_big, osb_pool, AO_T, wo_s, out, qt - 2)
        for mt in (NQ - 2, NQ - 1):
            _emit_wo(nc, ps_big, osb_pool, AO_T, wo_s, out, mt)

    if not nc.is_finalized():
        nc.finalize()
    return nc


_NC = None


def _get_nc():
    global _NC
    if _NC is None:
        _NC = build()
    return _NC


def _in_maps(hidden_states, Wq, Wk, Wv, Wo):
    import ml_dtypes

    bf = ml_dtypes.bfloat16
    hs = np.asarray(hidden_states, dtype=np.float32)
    Wq = np.asarray(Wq, dtype=np.float32)
    Wk = np.asarray(Wk, dtype=np.float32)
    Wv = np.asarray(Wv, dtype=np.float32)
    Wo = np.asarray(Wo, dtype=np.float32)
    maps = []
    for c in range(8):
        b, g = divmod(c, 4)
        sl = slice(512 * g, 512 * (g + 1))
        maps.append(
            {
                "ht": np.ascontiguousarray(hs[b].T).astype(bf),
                "wq": np.ascontiguousarray(Wq[:, sl]).astype(bf),
                "wk": np.ascontiguousarray(Wk[:, sl]).astype(bf),
                "wv": np.ascontiguousarray(Wv[:, sl]).astype(bf),
                "wo": np.ascontiguousarray(Wo[sl, :]).astype(bf),
            }
        )
    return maps


def _gather(results):
    outs = [np.asarray(results[c]["out"]).astype(np.float32) for c in range(8)]
    return np.stack(
        [outs[0] + outs[1] + outs[2] + outs[3],
         outs[4] + outs[5] + outs[6] + outs[7]]
    )


def run(in_maps, trace=False, **kw):
    nc = _get_nc()
    return run_bass_kernel_spmd(nc, in_maps, core_ids=list(range(8)), trace=trace, **kw)


def kernel(hidden_states, Wq, Wk, Wv, Wo):
    maps = _in_maps(hidden_states, Wq, Wk, Wv, Wo)
    res = run(maps)
    return _gather(res.results)
